# revision 16
# baseline (speedup 1.0000x reference)
"""Trainium2 Bass kernel for nn_BasicModel_4054449127788.

Quantum-circuit product-state model: per-(batch, qubit) single-qubit gate
chain (Rx/Rz/Rx + data-encoding Rx, 6 blocks), then Z^(x)n expectation of
the kron-folded wavefunction.

Math used on device: adjacent Rx gates commute and merge (Rx(a)Rx(b) =
Rx(a+b)), collapsing the 24-gate chain to 13 gates.  The Z^(x)n
expectation of a product state factorizes exactly:
    O_b = prod_q (|s_{b,q,0}|^2 - |s_{b,q,1}|^2)
which is numerically *closer* to the reference's f32 kron-fold + signed
sum than an independently-rounded fold replica would be (the fold's own
f32 cancellation noise dominates: ~2e-3 normwise).

Sharding: pure data parallelism — batch 32 split 4-per-core across 8
NeuronCores, no cross-core communication.

Layout on device: 80 partitions = (b_local=4) x (qubit=20), state kept as
4 f32 components [s0r, s0i, s1r, s1i] in the free dim.  Each gate is two
DVE instructions:
    tmp  = perm(S) * (sin * sign_pattern)      (tensor_tensor)
    S'   = (S * cos_perpartition) + tmp        (scalar_tensor_tensor)
where perm is free-dim reversal (Rx) or within-pair swap (Rz).
"""

import os
import numpy as np

_B = 32          # full batch
_Q = 20          # qubits
_NCORES = 8
_BL = _B // _NCORES   # batch per core = 4
_P = _BL * _Q         # partitions used = 80
_COLS = (0, 1, 2, 5, 6, 7)
_HALF_PI = float(np.pi / 2)

_CACHE = {}

# Exposed for test harnesses: exec time of the last traced run (ns).
LAST_EXEC_TIME_NS = None
LAST_RESULTS = None


def _build_nc():
    import concourse.bass as bass
    import concourse.mybir as mybir
    from concourse.tile import TileContext

    f32 = mybir.dt.float32
    ADD = mybir.AluOpType.add
    MULT = mybir.AluOpType.mult
    MIN = mybir.AluOpType.min
    MAX = mybir.AluOpType.max
    SUB = mybir.AluOpType.subtract
    SIN = mybir.ActivationFunctionType.Sin

    nc = bass.Bass("TRN2", target_bir_lowering=False, debug=False)

    # single packed input: cols 0..23 raw gate angles, 24..79 sign patterns
    # (one DMA -> one sem -> at most one DMA wait on any DVE instruction)
    inp = nc.dram_tensor("inp", [_P, 80], f32, kind="ExternalInput")
    st_out = nc.dram_tensor("state_out", [_P, 4], f32, kind="ExternalOutput")
    o_out = nc.dram_tensor("o_out", [_BL, 1], f32, kind="ExternalOutput")

    with TileContext(nc) as tc:
        with (
            tc.tile_pool(name="cst", bufs=1) as cst,
            tc.tile_pool(name="ping", bufs=2) as ping,
        ):
            IN = cst.tile([_P, 80], f32, tag="IN")
            nc.sync.dma_start(out=IN[:], in_=inp[:])
            A = IN[:, 0:24]
            PT = IN[:, 24:80]

            # ---- merge the 24 raw gate angles into 13 -------------------
            # gate order g=0..12: Rx(a0), then 6x [Rz(beta_i), Rx(alpha_i)]
            #   a0      = raw[0]
            #   beta_i  = raw[4i-3]                      (g = 2i-1)
            #   alpha_i = raw[4i-2]+raw[4i-1]+raw[4i]    (g = 2i, i=1..5)
            #   alpha_6 = raw[22]+raw[23]                (g = 12)
            AN = cst.tile([_P, 13], f32, tag="AN")
            T5 = cst.tile([_P, 5], f32, tag="T5")
            nc.vector.tensor_copy(AN[:, 0:1], A[:, 0:1])
            nc.vector.tensor_copy(AN[:, 1:12:2], A[:, 1:22:4])
            nc.vector.tensor_tensor(T5[:], A[:, 2:19:4], A[:, 3:20:4], ADD)
            nc.vector.tensor_tensor(AN[:, 2:11:2], T5[:], A[:, 4:21:4], ADD)
            nc.vector.tensor_tensor(AN[:, 12:13], A[:, 22:23], A[:, 23:24], ADD)

            # ---- cos/sin of half-angles --------------------------------
            # HW Sin needs args in [-pi, pi]; merged angles can exceed it.
            # Range-reduce t -> t - 2pi*round(t/2pi) using the f32
            # magic-constant round (x + 1.5*2^23 - 1.5*2^23), then clamp.
            PI = float(np.pi)
            MAGIC = float(1.5 * 2.0 ** 23)
            AH = cst.tile([_P, 26], f32, tag="AH")   # [sin args | cos args]
            nc.vector.tensor_scalar(AH[:, 0:13], AN[:], 0.5, None, MULT)
            nc.vector.tensor_scalar(AH[:, 13:26], AH[:, 0:13], _HALF_PI, None, ADD)
            U = cst.tile([_P, 26], f32, tag="U")
            R = cst.tile([_P, 26], f32, tag="R")
            nc.vector.tensor_scalar(U[:], AH[:], 1.0 / (2.0 * PI), None, MULT)
            nc.vector.tensor_scalar(R[:], U[:], MAGIC, None, ADD)
            nc.vector.tensor_scalar(R[:], R[:], MAGIC, None, SUB)
            RD = cst.tile([_P, 26], f32, tag="RD")
            nc.vector.scalar_tensor_tensor(
                RD[:], R[:], -2.0 * PI, AH[:], MULT, ADD
            )
            nc.vector.tensor_scalar(RD[:], RD[:], PI, None, MIN)
            nc.vector.tensor_scalar(RD[:], RD[:], -PI, None, MAX)
            CS = cst.tile([_P, 26], f32, tag="CS")   # [sin | cos] results
            nc.scalar.activation(CS[:], RD[:], SIN, bias=0.0, scale=1.0)
            SN = CS[:, 0:13]
            C = CS[:, 13:26]

            # ---- sin * per-component sign pattern ----------------------
            # comp-major layout SS[p, c*13 + g] keeps every AP here fully
            # contiguous (3-level-AP TT ops run out of sync-wait slots in
            # the S3S3D3 encoding).
            SS = cst.tile([_P, 52], f32, tag="SS")
            for c in range(4):
                nc.vector.tensor_tensor(
                    SS[:, 13 * c:13 * c + 13], SN,
                    PT[:, 13 * c:13 * c + 13], MULT,
                )

            # Absorb the cross-engine wait on C into a single-dependency
            # DVE op: the first gate's scalar_tensor_tensor would otherwise
            # carry two sync waits (ACT sem + same-engine), which the
            # TensorScalarPtr encoding cannot hold.
            CT = cst.tile([_P, 1], f32, tag="CT")
            nc.vector.tensor_copy(CT[:], C[:, 12:13])

            # ---- gate chain --------------------------------------------
            S = ping.tile([_P, 4], f32, tag="st")
            nc.vector.memset(S[:, 0:1], 1.0)
            nc.vector.memset(S[:, 1:4], 0.0)
            for g in range(13):
                TMP = ping.tile([_P, 4], f32, tag="tmp")
                SNEW = ping.tile([_P, 4], f32, tag="st")
                ss_g = SS[:, g:52:13]  # comps [0..3] of gate g, stride 13
                if g % 2 == 0:
                    # Rx: perm = component reversal [s1i, s1r, s0i, s0r]
                    nc.vector.tensor_tensor(
                        TMP[:], S[:][:, ::-1], ss_g, MULT
                    )
                else:
                    # Rz: perm = within-pair swap [s0i, s0r, s1i, s1r]
                    perm = S[:].rearrange("p (a b) -> p a b", b=2)[:, :, ::-1]
                    tmp_v = TMP[:].rearrange("p (a b) -> p a b", b=2)
                    ss_v = ss_g.rearrange("p (a b) -> p a b", b=2)
                    nc.vector.tensor_tensor(tmp_v, perm, ss_v, MULT)
                nc.vector.scalar_tensor_tensor(
                    SNEW[:], S[:], C[:, g:g + 1], TMP[:], MULT, ADD
                )
                S = SNEW

            # ---- outputs -----------------------------------------------
            nc.sync.dma_start(out=st_out[:], in_=S[:])

            # z = s0r^2 + s0i^2 - s1r^2 - s1i^2  per (b, q) partition
            SQ = cst.tile([_P, 4], f32, tag="SQ")
            T2 = cst.tile([_P, 2], f32, tag="T2")
            Z = cst.tile([_P, 1], f32, tag="Z")
            nc.vector.tensor_tensor(SQ[:], S[:], S[:], MULT)
            nc.vector.tensor_tensor(T2[:], SQ[:, 0:2], SQ[:, 2:4], SUB)
            nc.vector.tensor_tensor(Z[:], T2[:, 0:1], T2[:, 1:2], ADD)

            # transpose z (80,1)->(4,20): partition (b,q) -> row b, col q
            ZT = cst.tile([_BL, _Q], f32, tag="ZT")
            nc.sync.dma_start(out=ZT[:], in_=Z[:])

            # product over the 20 qubits (halving tree)
            M1 = cst.tile([_BL, 10], f32, tag="M1")
            M2 = cst.tile([_BL, 5], f32, tag="M2")
            M3 = cst.tile([_BL, 2], f32, tag="M3")
            M4 = cst.tile([_BL, 1], f32, tag="M4")
            OO = cst.tile([_BL, 1], f32, tag="OO")
            nc.vector.tensor_tensor(M1[:], ZT[:, 0:10], ZT[:, 10:20], MULT)
            nc.vector.tensor_tensor(M2[:], M1[:, 0:5], M1[:, 5:10], MULT)
            nc.vector.tensor_tensor(M3[:], M2[:, 0:2], M2[:, 2:4], MULT)
            nc.vector.tensor_tensor(M4[:], M3[:, 0:1], M3[:, 1:2], MULT)
            nc.vector.tensor_tensor(OO[:], M4[:], M2[:, 4:5], MULT)
            nc.sync.dma_start(out=o_out[:], in_=OO[:])

    _split_multi_waits(nc)
    return nc


def _split_multi_waits(nc, max_waits=1):
    """The walrus build in this toolchain allows at most one embedded sync
    wait per instruction; Tile can emit more (e.g. the kernel-tail drain).
    Hoist excess waits into single-wait NoOps on the same engine queue."""
    import concourse.mybir as mybir

    n = 0
    for bb in nc.m.functions[0].blocks:
        out_list = []
        changed = False
        for ins in bb.instructions:
            si = getattr(ins, "sync_info", None)
            waits = list(si.on_wait) if (si and si.on_wait) else []
            if len(waits) > max_waits:
                for w in waits[:-max_waits]:
                    nop = mybir.InstNoOp(name=f"nop-wait-{n}")
                    n += 1
                    nop.engine = ins.engine
                    nop.sync_info = mybir.SyncInfo(on_wait=[w], on_update=[])
                    nc.register_instruction(nop, overwrite=True)
                    out_list.append(nop)
                ins.sync_info = mybir.SyncInfo(
                    on_wait=waits[-max_waits:], on_update=list(si.on_update)
                )
                changed = True
            out_list.append(ins)
        if changed:
            bb.instructions = out_list


def _pattern_input():
    """(P, 56) constant: per-gate sign patterns in comp-major layout
    [c*13 + g] (cols 0..51) + the z-reduction pattern [1,1,-1,-1]
    (cols 52..55)."""
    pat = np.empty((13, 4), np.float32)
    for g in range(13):
        pat[g] = (1, -1, 1, -1) if g % 2 == 0 else (1, -1, -1, 1)
    row = np.concatenate([pat.T.reshape(-1), np.array([1, 1, -1, -1], np.float32)])
    return np.broadcast_to(row, (_P, 56)).copy()


def _pack_angles(x, w):
    """(B, Q, 24) raw gate angles in application order."""
    ang = np.empty((_B, _Q, 24), np.float32)
    for i in range(6):
        ang[:, :, 4 * i + 0] = w[i, 0]
        ang[:, :, 4 * i + 1] = w[i, 1]
        ang[:, :, 4 * i + 2] = w[i, 2]
        ang[:, :, 4 * i + 3] = x[:, _COLS[i], :]
    return ang


def kernel(x, weights):
    global LAST_EXEC_TIME_NS, LAST_RESULTS
    from concourse.bass_utils import run_bass_kernel_spmd

    x = np.ascontiguousarray(np.asarray(x, np.float32))
    w = np.ascontiguousarray(np.asarray(weights, np.float32))

    if "nc" not in _CACHE:
        _CACHE["nc"] = _build_nc()
        _CACHE["pat"] = _pattern_input()
    nc = _CACHE["nc"]
    pat = _CACHE["pat"]

    ang = _pack_angles(x, w)  # (B, Q, 24)
    in_maps = []
    for c in range(_NCORES):
        packed = np.empty((_P, 80), np.float32)
        packed[:, 0:24] = ang[c * _BL:(c + 1) * _BL].reshape(_P, 24)
        packed[:, 24:80] = pat
        in_maps.append({"inp": packed})

    trace = os.environ.get("KERNEL_TRACE", "0") == "1"
    res = run_bass_kernel_spmd(nc, in_maps, list(range(_NCORES)), trace=trace)
    LAST_EXEC_TIME_NS = res.exec_time_ns
    LAST_RESULTS = res

    state = np.empty((_B, _Q, 2), np.complex64)
    O = np.empty((_B, 1, 1), np.complex64)
    for c in range(_NCORES):
        st = np.asarray(res.results[c]["state_out"], np.float32)
        st = st.reshape(_BL, _Q, 2, 2)
        state[c * _BL:(c + 1) * _BL] = st[..., 0] + 1j * st[..., 1]
        oo = np.asarray(res.results[c]["o_out"], np.float32).reshape(_BL)
        O[c * _BL:(c + 1) * _BL, 0, 0] = oo.astype(np.complex64)

    return state.reshape(_B, _Q, 1, 2, 1), O


# revision 20
# speedup vs baseline: 1.1284x; 1.1284x over previous
"""Trainium2 Bass kernel for nn_BasicModel_4054449127788.

Quantum-circuit product-state model: per-(batch, qubit) single-qubit gate
chain (Rx/Rz/Rx + data-encoding Rx, 6 blocks), then Z^(x)n expectation of
the kron-folded wavefunction.

Math used on device: adjacent Rx gates commute and merge (Rx(a)Rx(b) =
Rx(a+b)), collapsing the 24-gate chain to 13 gates.  The Z^(x)n
expectation of a product state factorizes exactly:
    O_b = prod_q (|s_{b,q,0}|^2 - |s_{b,q,1}|^2)
which is numerically *closer* to the reference's f32 kron-fold + signed
sum than an independently-rounded fold replica would be (the fold's own
f32 cancellation noise dominates: ~2e-3 normwise).

Sharding: pure data parallelism — batch 32 split 4-per-core across 8
NeuronCores, no cross-core communication.

Layout on device: 128 partitions = (b_local=4) x (32-lane quadrant), with
qubit q = 0..19 at partition b*32 + q (lanes 20..31 idle).  The state is
4 f32 components [s0r, s0i, s1r, s1i] in the free dim.  Each gate is two
DVE instructions:
    tmp  = perm(S) * (sin * sign_pattern)      (tensor_tensor)
    S'   = (S * cos_perpartition) + tmp        (scalar_tensor_tensor)
where perm is free-dim reversal (Rx) or within-pair swap (Rz).  The
product over the 20 qubits runs in-layout with stream_shuffle quadrant
rotations (no transpose DMA), and O rides in the same output DMA as the
state.
"""

import os
import numpy as np

_B = 32          # full batch
_Q = 20          # qubits
_NCORES = 8
_BL = _B // _NCORES   # batch per core = 4
_P = 128              # partitions: b_local * 32 + q
_COLS = (0, 1, 2, 5, 6, 7)
_HALF_PI = float(np.pi / 2)

_CACHE = {}

# Exposed for test harnesses: exec time of the last traced run (ns).
LAST_EXEC_TIME_NS = None
LAST_RESULTS = None


def _build_nc():
    import concourse.bass as bass
    import concourse.mybir as mybir
    from concourse.tile import TileContext

    f32 = mybir.dt.float32
    ADD = mybir.AluOpType.add
    MULT = mybir.AluOpType.mult
    MIN = mybir.AluOpType.min
    MAX = mybir.AluOpType.max
    SUB = mybir.AluOpType.subtract
    SIN = mybir.ActivationFunctionType.Sin

    nc = bass.Bass("TRN2", target_bir_lowering=False, debug=False)

    # single packed input: cols 0..23 raw gate angles, 24..79 sign patterns
    inp = nc.dram_tensor("inp", [_P, 80], f32, kind="ExternalInput")
    # single packed output: cols 0..3 state comps, col 4 = O at lanes b*32
    outp = nc.dram_tensor("outp", [_P, 5], f32, kind="ExternalOutput")

    def rot_mask(n):
        return [(i + n) % 32 for i in range(32)]

    with TileContext(nc) as tc:
        with (
            tc.tile_pool(name="cst", bufs=1) as cst,
            tc.tile_pool(name="ping", bufs=2) as ping,
        ):
            IN = cst.tile([_P, 80], f32, tag="IN")
            nc.sync.dma_start(out=IN[:], in_=inp[:])
            A = IN[:, 0:24]
            PT = IN[:, 24:80]

            # one workspace tile for all DVE scratch (fewer tile sems ->
            # shorter kernel-tail semaphore-reset sequence)
            WS = cst.tile([_P, 200], f32, tag="WS")
            AN = WS[:, 0:13]
            T5 = WS[:, 13:18]
            AH = WS[:, 18:44]    # [sin half-angles | cos half-angles]
            U = WS[:, 44:70]
            R = WS[:, 70:96]
            RD = WS[:, 96:122]
            CS = WS[:, 122:148]  # [sin | cos] results
            SS = WS[:, 148:200]  # sin * per-comp sign pattern, comp-major

            OUT = cst.tile([_P, 5], f32, tag="OUT")

            # ---- merge the 24 raw gate angles into 13 ------------------
            # gate order g=0..12: Rx(a0), then 6x [Rz(beta_i), Rx(alpha_i)]
            nc.vector.tensor_copy(AN[:, 0:1], A[:, 0:1])
            nc.vector.tensor_copy(AN[:, 1:12:2], A[:, 1:22:4])
            nc.vector.tensor_tensor(T5[:, :], A[:, 2:19:4], A[:, 3:20:4], ADD)
            nc.vector.tensor_tensor(AN[:, 2:11:2], T5[:, :], A[:, 4:21:4], ADD)
            nc.vector.tensor_tensor(AN[:, 12:13], A[:, 22:23], A[:, 23:24], ADD)

            # ---- cos/sin of half-angles --------------------------------
            # HW Sin needs args in [-pi, pi]; merged angles can exceed it.
            # Range-reduce t -> t - 2pi*round(t/2pi) using the f32
            # magic-constant round (x + 1.5*2^23 - 1.5*2^23), then clamp.
            PI = float(np.pi)
            MAGIC = float(1.5 * 2.0 ** 23)
            nc.vector.tensor_scalar(AH[:, 0:13], AN[:, :], 0.5, None, MULT)
            nc.vector.tensor_scalar(
                AH[:, 13:26], AN[:, :], 0.5, _HALF_PI, MULT, ADD
            )
            nc.vector.tensor_scalar(
                U[:, :], AH[:, :], 1.0 / (2.0 * PI), MAGIC, MULT, ADD
            )
            nc.vector.tensor_scalar(R[:, :], U[:, :], MAGIC, None, SUB)
            nc.vector.scalar_tensor_tensor(
                RD[:, :], R[:, :], -2.0 * PI, AH[:, :], MULT, ADD
            )
            nc.vector.tensor_scalar(RD[:, :], RD[:, :], PI, -PI, MIN, MAX)
            nc.scalar.activation(CS[:, :], RD[:, :], SIN, bias=0.0, scale=1.0)
            SN = CS[:, 0:13]
            C = CS[:, 13:26]

            # ---- sin * per-component sign pattern ----------------------
            # comp-major layout SS[p, c*13 + g] keeps every AP contiguous
            for c in range(4):
                nc.vector.tensor_tensor(
                    SS[:, 13 * c:13 * c + 13], SN,
                    PT[:, 13 * c:13 * c + 13], MULT,
                )

            # ---- gate chain --------------------------------------------
            # g0 = Rx(a0) on |0>: state = (cos, 0, 0, -sin) directly
            S = ping.tile([_P, 4], f32, tag="st")
            nc.vector.tensor_copy(S[:, 0:1], C[:, 0:1])
            nc.vector.memset(S[:, 1:3], 0.0)
            nc.vector.tensor_copy(S[:, 3:4], SS[:, 39:40])
            for g in range(1, 13):
                TMP = ping.tile([_P, 4], f32, tag="tmp")
                if g == 12:
                    SNEW = OUT[:, 0:4]
                else:
                    SNEW_T = ping.tile([_P, 4], f32, tag="st")
                    SNEW = SNEW_T[:]
                ss_g = SS[:, g:52:13]  # comps [0..3] of gate g, stride 13
                if g % 2 == 0:
                    # Rx: perm = component reversal [s1i, s1r, s0i, s0r]
                    nc.vector.tensor_tensor(TMP[:], S[:][:, ::-1], ss_g, MULT)
                else:
                    # Rz: perm = within-pair swap [s0i, s0r, s1i, s1r]
                    perm = S[:].rearrange("p (a b) -> p a b", b=2)[:, :, ::-1]
                    tmp_v = TMP[:].rearrange("p (a b) -> p a b", b=2)
                    ss_v = ss_g.rearrange("p (a b) -> p a b", b=2)
                    nc.vector.tensor_tensor(tmp_v, perm, ss_v, MULT)
                nc.vector.scalar_tensor_tensor(
                    SNEW, S[:], C[:, g:g + 1], TMP[:], MULT, ADD
                )
                S = SNEW

            SF = OUT[:, 0:4]

            # ---- z = s0r^2 + s0i^2 - s1r^2 - s1i^2 per (b, q) lane -----
            SQ = WS[:, 0:4]      # reuse workspace columns
            T2 = WS[:, 4:6]
            Z = WS[:, 6:7]
            nc.vector.tensor_tensor(SQ, SF, SF, MULT)
            nc.vector.tensor_tensor(T2, SQ[:, 0:2], SQ[:, 2:4], SUB)
            nc.vector.tensor_tensor(Z, T2[:, 0:1], T2[:, 1:2], ADD)

            # ---- product over 20 qubit lanes via quadrant rotations ----
            SH = WS[:, 7:8]
            P1 = WS[:, 8:9]
            P2 = WS[:, 9:10]
            P3 = WS[:, 10:11]
            P4 = WS[:, 11:12]
            nc.vector.stream_shuffle(SH, Z, rot_mask(10))
            nc.vector.tensor_tensor(P1, Z, SH, MULT)        # lanes 0..9
            nc.vector.stream_shuffle(SH, P1, rot_mask(5))
            nc.vector.tensor_tensor(P2, P1, SH, MULT)       # lanes 0..4
            nc.vector.stream_shuffle(SH, P2, rot_mask(2))
            nc.vector.tensor_tensor(P3, P2, SH, MULT)       # lanes 0..1
            nc.vector.stream_shuffle(SH, P3, rot_mask(1))
            nc.vector.tensor_tensor(P4, P3, SH, MULT)       # lane 0
            nc.vector.stream_shuffle(SH, P2, rot_mask(4))
            nc.vector.tensor_tensor(OUT[:, 4:5], P4, SH, MULT)

            nc.sync.dma_start(out=outp[:], in_=OUT[:])

    _split_multi_waits(nc)
    _hoist_input_dma(nc)
    return nc


def _split_multi_waits(nc, max_waits=1):
    """The walrus build in this toolchain allows at most one embedded sync
    wait per instruction; Tile can emit more (e.g. the kernel-tail drain).
    Hoist excess waits into single-wait NoOps on the same engine queue."""
    import concourse.mybir as mybir

    n = 0
    for bb in nc.m.functions[0].blocks:
        out_list = []
        changed = False
        for ins in bb.instructions:
            si = getattr(ins, "sync_info", None)
            waits = list(si.on_wait) if (si and si.on_wait) else []
            if len(waits) > max_waits:
                for w in waits[:-max_waits]:
                    nop = mybir.InstNoOp(name=f"nop-wait-{n}")
                    n += 1
                    nop.engine = ins.engine
                    nop.sync_info = mybir.SyncInfo(on_wait=[w], on_update=[])
                    nc.register_instruction(nop, overwrite=True)
                    out_list.append(nop)
                ins.sync_info = mybir.SyncInfo(
                    on_wait=waits[-max_waits:], on_update=list(si.on_update)
                )
                changed = True
            out_list.append(ins)
        if changed:
            bb.instructions = out_list


def _hoist_input_dma(nc):
    """Move the (wait-free) input DMA to the front of the program so the
    transfer overlaps the framework preamble barriers instead of queuing
    behind them (~2us saved)."""
    blocks = nc.m.functions[0].blocks
    if len(blocks) < 2:
        return
    tile_bb = blocks[1]
    insts = list(tile_bb.instructions)
    dma = None
    for ins in insts:
        if type(ins).__name__ == "InstDMACopy":
            si = getattr(ins, "sync_info", None)
            if not (si and si.on_wait):
                dma = ins
            break
    if dma is None:
        return
    insts.remove(dma)
    tile_bb.instructions = insts
    main = list(blocks[0].instructions)
    main.insert(1, dma)
    blocks[0].instructions = main


def _pattern_input():
    """(56,) constant row: per-gate sign patterns in comp-major layout
    [c*13 + g] (cols 0..51) + 4 spare cols."""
    pat = np.empty((13, 4), np.float32)
    for g in range(13):
        pat[g] = (1, -1, 1, -1) if g % 2 == 0 else (1, -1, -1, 1)
    return np.concatenate([pat.T.reshape(-1), np.zeros(4, np.float32)])


def _pack_angles(x, w):
    """(B, Q, 24) raw gate angles in application order."""
    ang = np.empty((_B, _Q, 24), np.float32)
    for i in range(6):
        ang[:, :, 4 * i + 0] = w[i, 0]
        ang[:, :, 4 * i + 1] = w[i, 1]
        ang[:, :, 4 * i + 2] = w[i, 2]
        ang[:, :, 4 * i + 3] = x[:, _COLS[i], :]
    return ang


def _pack_core_input(ang, pat_row, c):
    packed = np.zeros((_P, 80), np.float32)
    for b in range(_BL):
        packed[b * 32:b * 32 + _Q, 0:24] = ang[c * _BL + b]
    packed[:, 24:80] = pat_row
    return packed


def kernel(x, weights):
    global LAST_EXEC_TIME_NS, LAST_RESULTS
    from concourse.bass_utils import run_bass_kernel_spmd

    x = np.ascontiguousarray(np.asarray(x, np.float32))
    w = np.ascontiguousarray(np.asarray(weights, np.float32))

    if "nc" not in _CACHE:
        _CACHE["nc"] = _build_nc()
        _CACHE["pat"] = _pattern_input()
    nc = _CACHE["nc"]
    pat_row = _CACHE["pat"]

    ang = _pack_angles(x, w)  # (B, Q, 24)
    in_maps = [
        {"inp": _pack_core_input(ang, pat_row, c)} for c in range(_NCORES)
    ]

    trace = os.environ.get("KERNEL_TRACE", "0") == "1"
    res = run_bass_kernel_spmd(nc, in_maps, list(range(_NCORES)), trace=trace)
    LAST_EXEC_TIME_NS = res.exec_time_ns
    LAST_RESULTS = res

    state = np.empty((_B, _Q, 2), np.complex64)
    O = np.empty((_B, 1, 1), np.complex64)
    for c in range(_NCORES):
        o = np.asarray(res.results[c]["outp"], np.float32)  # (128, 5)
        for b in range(_BL):
            st = o[b * 32:b * 32 + _Q, 0:4].reshape(_Q, 2, 2)
            state[c * _BL + b] = st[..., 0] + 1j * st[..., 1]
            O[c * _BL + b, 0, 0] = np.complex64(o[b * 32, 4])

    return state.reshape(_B, _Q, 1, 2, 1), O


# revision 28
# speedup vs baseline: 1.1450x; 1.0147x over previous
"""Trainium2 Bass kernel for nn_BasicModel_4054449127788.

Quantum-circuit product-state model: per-(batch, qubit) single-qubit gate
chain (Rx/Rz/Rx + data-encoding Rx, 6 blocks), then Z^(x)n expectation of
the kron-folded wavefunction.

Math used on device: adjacent Rx gates commute and merge (Rx(a)Rx(b) =
Rx(a+b)), collapsing the 24-gate chain to 13 gates.  The Z^(x)n
expectation of a product state factorizes exactly:
    O_b = prod_q (|s_{b,q,0}|^2 - |s_{b,q,1}|^2)
which is numerically *closer* to the reference's f32 kron-fold + signed
sum than an independently-rounded fold replica would be (the fold's own
f32 cancellation noise dominates: ~2e-3 normwise).

Sharding: pure data parallelism — batch 32 split 4-per-core across 8
NeuronCores, no cross-core communication.

Layout on device: 128 partitions = (b_local=4) x (32-lane quadrant), with
qubit q = 0..19 at partition b*32 + q (lanes 20..31 idle).  The state is
4 f32 components [s0r, s0i, s1r, s1i] in the free dim.  Each gate is two
DVE instructions:
    tmp  = perm(S) * (sin * sign_pattern)      (tensor_tensor)
    S'   = (S * cos_perpartition) + tmp        (scalar_tensor_tensor)
where perm is free-dim reversal (Rx) or within-pair swap (Rz).  The
product over the 20 qubits runs in-layout with stream_shuffle quadrant
rotations (no transpose DMA), and O rides in the same output DMA as the
state.
"""

import os
import numpy as np

_B = 32          # full batch
_Q = 20          # qubits
_NCORES = 8
_BL = _B // _NCORES   # batch per core = 4
_P = 128              # partitions: b_local * 32 + q
_COLS = (0, 1, 2, 5, 6, 7)
_HALF_PI = float(np.pi / 2)

_CACHE = {}

# Exposed for test harnesses: exec time of the last traced run (ns).
LAST_EXEC_TIME_NS = None
LAST_RESULTS = None


def _build_nc():
    import concourse.bass as bass
    import concourse.mybir as mybir
    from concourse.tile import TileContext

    f32 = mybir.dt.float32
    ADD = mybir.AluOpType.add
    MULT = mybir.AluOpType.mult
    MIN = mybir.AluOpType.min
    MAX = mybir.AluOpType.max
    SUB = mybir.AluOpType.subtract
    SIN = mybir.ActivationFunctionType.Sin

    nc = bass.Bass("TRN2", target_bir_lowering=False, debug=False)

    # single packed input: cols 0..24 raw gate angles (col 24 zero pad so
    # every merged alpha is a sum of three stride-4 columns), cols 25..80
    # sign patterns
    inp = nc.dram_tensor("inp", [_P, 81], f32, kind="ExternalInput")
    # single packed output: cols 0..3 state comps, col 4 = O at lanes b*32
    outp = nc.dram_tensor("outp", [_P, 5], f32, kind="ExternalOutput")

    def rot_mask(n):
        return [(i + n) % 32 for i in range(32)]

    with TileContext(nc) as tc:
        with (
            tc.tile_pool(name="cst", bufs=1) as cst,
            tc.tile_pool(name="ping", bufs=2) as ping,
        ):
            IN = cst.tile([_P, 81], f32, tag="IN")
            nc.sync.dma_start(out=IN[:], in_=inp[:])
            A = IN[:, 0:25]
            PT = IN[:, 25:81]

            # one workspace tile for all DVE scratch (fewer tile sems ->
            # shorter kernel-tail semaphore-reset sequence)
            WS = cst.tile([_P, 200], f32, tag="WS")
            T6 = WS[:, 0:6]
            T6b = WS[:, 6:12]
            AH = WS[:, 18:44]    # [sin half-angles | cos half-angles]
            U = WS[:, 44:70]
            R = WS[:, 70:96]
            RD = WS[:, 96:122]
            CS = WS[:, 122:148]  # [sin | cos] results
            SS = WS[:, 148:200]  # sin * per-comp sign pattern, comp-major

            OUT = cst.tile([_P, 5], f32, tag="OUT")

            # ---- merge the 24 raw gate angles into 13 half-angles ------
            # gate order g=0..12: Rx(a0), then 6x [Rz(beta_i), Rx(alpha_i)]
            # alpha_i = A[4i-2] + A[4i-1] + A[4i]  (A[24] = 0 pad makes the
            # i=6 group uniform), beta_i = A[4i-3], a0 = A[0].
            PI = float(np.pi)
            MAGIC = float(1.5 * 2.0 ** 23)
            nc.vector.tensor_tensor(T6b[:, :], A[:, 2:23:4], A[:, 3:24:4], ADD)
            nc.vector.tensor_tensor(T6[:, :], T6b[:, :], A[:, 4:25:4], ADD)
            nc.vector.tensor_scalar(AH[:, 0:1], A[:, 0:1], 0.5, None, MULT)
            nc.vector.tensor_scalar(AH[:, 1:12:2], A[:, 1:22:4], 0.5, None, MULT)
            nc.vector.tensor_scalar(AH[:, 2:13:2], T6[:, :], 0.5, None, MULT)

            # ---- cos/sin of half-angles --------------------------------
            # HW Sin needs args in [-pi, pi]; merged angles can exceed it.
            # Range-reduce t -> t - 2pi*round(t/2pi) using the f32
            # magic-constant round (x + 1.5*2^23 - 1.5*2^23), then clamp.
            nc.vector.tensor_scalar(
                AH[:, 13:26], AH[:, 0:13], _HALF_PI, None, ADD
            )
            nc.vector.tensor_scalar(
                U[:, :], AH[:, :], 1.0 / (2.0 * PI), MAGIC, MULT, ADD
            )
            nc.vector.tensor_scalar(R[:, :], U[:, :], MAGIC, None, SUB)
            nc.vector.scalar_tensor_tensor(
                RD[:, :], R[:, :], -2.0 * PI, AH[:, :], MULT, ADD
            )
            nc.vector.tensor_scalar(RD[:, :], RD[:, :], PI, -PI, MIN, MAX)
            nc.scalar.activation(CS[:, :], RD[:, :], SIN, bias=0.0, scale=1.0)
            SN = CS[:, 0:13]
            C = CS[:, 13:26]

            # ---- sin * per-component sign pattern ----------------------
            # comp-major layout SS[p, c*13 + g]; one TT with a broadcast AP
            sn_b = SN.unsqueeze(1).broadcast_to([_P, 4, 13])
            pt_v = PT[:, 0:52].rearrange("p (c g) -> p c g", g=13)
            ss_v = SS.rearrange("p (c g) -> p c g", g=13)
            nc.vector.tensor_tensor(ss_v, sn_b, pt_v, MULT)

            # ---- gate chain --------------------------------------------
            # g0 = Rx(a0) on |0>: state = (cos, 0, 0, -sin) directly.
            # cos0 is WS col 122+13=135 and -sin0 is WS col 148+39=187, so
            # one strided copy fills comps {0, 3}.
            S = ping.tile([_P, 4], f32, tag="st")
            nc.vector.memset(S[:, 1:3], 0.0)
            nc.vector.tensor_copy(S[:, 0:4:3], WS[:, 135:188:52])
            for g in range(1, 13):
                TMP = ping.tile([_P, 4], f32, tag="tmp")
                if g == 12:
                    SNEW = OUT[:, 0:4]
                else:
                    SNEW_T = ping.tile([_P, 4], f32, tag="st")
                    SNEW = SNEW_T[:]
                ss_g = SS[:, g:52:13]  # comps [0..3] of gate g, stride 13
                if g % 2 == 0:
                    # Rx: perm = component reversal [s1i, s1r, s0i, s0r]
                    nc.vector.tensor_tensor(TMP[:], S[:][:, ::-1], ss_g, MULT)
                else:
                    # Rz: perm = within-pair swap [s0i, s0r, s1i, s1r]
                    perm = S[:].rearrange("p (a b) -> p a b", b=2)[:, :, ::-1]
                    tmp_v = TMP[:].rearrange("p (a b) -> p a b", b=2)
                    ss_v = ss_g.rearrange("p (a b) -> p a b", b=2)
                    nc.vector.tensor_tensor(tmp_v, perm, ss_v, MULT)
                nc.vector.scalar_tensor_tensor(
                    SNEW, S[:], C[:, g:g + 1], TMP[:], MULT, ADD
                )
                S = SNEW

            SF = OUT[:, 0:4]
            # ship the state while the expectation is still being computed
            nc.sync.dma_start(out=outp[:, 0:4], in_=SF)

            # ---- z = s0r^2 + s0i^2 - s1r^2 - s1i^2 per (b, q) lane -----
            SQ = WS[:, 0:4]      # reuse workspace columns
            T2 = WS[:, 4:6]
            Z = WS[:, 6:7]
            nc.vector.tensor_tensor(SQ, SF, SF, MULT)
            nc.vector.tensor_tensor(T2, SQ[:, 0:2], SQ[:, 2:4], SUB)
            nc.vector.tensor_tensor(Z, T2[:, 0:1], T2[:, 1:2], ADD)

            # ---- product over 20 qubit lanes via quadrant rotations ----
            SH = WS[:, 7:8]
            P1 = WS[:, 8:9]
            P2 = WS[:, 9:10]
            P3 = WS[:, 10:11]
            P4 = WS[:, 11:12]
            nc.vector.stream_shuffle(SH, Z, rot_mask(10))
            nc.vector.tensor_tensor(P1, Z, SH, MULT)        # lanes 0..9
            nc.vector.stream_shuffle(SH, P1, rot_mask(5))
            nc.vector.tensor_tensor(P2, P1, SH, MULT)       # lanes 0..4
            nc.vector.stream_shuffle(SH, P2, rot_mask(2))
            nc.vector.tensor_tensor(P3, P2, SH, MULT)       # lanes 0..1
            nc.vector.stream_shuffle(SH, P3, rot_mask(1))
            nc.vector.tensor_tensor(P4, P3, SH, MULT)       # lane 0
            nc.vector.stream_shuffle(SH, P2, rot_mask(4))
            nc.vector.tensor_tensor(OUT[:, 4:5], P4, SH, MULT)

            nc.sync.dma_start(out=outp[:, 4:5], in_=OUT[:, 4:5])

    _split_multi_waits(nc)
    _hoist_input_dma(nc)
    return nc


def _split_multi_waits(nc, max_waits=1):
    """The walrus build in this toolchain allows at most one embedded sync
    wait per instruction; Tile can emit more (e.g. the kernel-tail drain).
    Hoist excess waits into single-wait NoOps on the same engine queue."""
    import concourse.mybir as mybir

    n = 0
    for bb in nc.m.functions[0].blocks:
        out_list = []
        changed = False
        for ins in bb.instructions:
            si = getattr(ins, "sync_info", None)
            waits = list(si.on_wait) if (si and si.on_wait) else []
            if len(waits) > max_waits:
                for w in waits[:-max_waits]:
                    nop = mybir.InstNoOp(name=f"nop-wait-{n}")
                    n += 1
                    nop.engine = ins.engine
                    nop.sync_info = mybir.SyncInfo(on_wait=[w], on_update=[])
                    nc.register_instruction(nop, overwrite=True)
                    out_list.append(nop)
                ins.sync_info = mybir.SyncInfo(
                    on_wait=waits[-max_waits:], on_update=list(si.on_update)
                )
                changed = True
            out_list.append(ins)
        if changed:
            bb.instructions = out_list


def _hoist_input_dma(nc):
    """Move the (wait-free) input DMA to the front of the program so the
    transfer overlaps the framework preamble barriers instead of queuing
    behind them (~2us saved)."""
    blocks = nc.m.functions[0].blocks
    if len(blocks) < 2:
        return
    tile_bb = blocks[1]
    insts = list(tile_bb.instructions)
    dma = None
    for ins in insts:
        if type(ins).__name__ == "InstDMACopy":
            si = getattr(ins, "sync_info", None)
            if not (si and si.on_wait):
                dma = ins
            break
    if dma is None:
        return
    insts.remove(dma)
    tile_bb.instructions = insts
    main = list(blocks[0].instructions)
    main.insert(1, dma)
    blocks[0].instructions = main


def _pattern_input():
    """(56,) constant row: per-gate sign patterns in comp-major layout
    [c*13 + g] (cols 0..51) + 4 spare cols."""
    pat = np.empty((13, 4), np.float32)
    for g in range(13):
        pat[g] = (1, -1, 1, -1) if g % 2 == 0 else (1, -1, -1, 1)
    return np.concatenate([pat.T.reshape(-1), np.zeros(4, np.float32)])


def _pack_angles(x, w):
    """(B, Q, 24) raw gate angles in application order."""
    ang = np.empty((_B, _Q, 24), np.float32)
    for i in range(6):
        ang[:, :, 4 * i + 0] = w[i, 0]
        ang[:, :, 4 * i + 1] = w[i, 1]
        ang[:, :, 4 * i + 2] = w[i, 2]
        ang[:, :, 4 * i + 3] = x[:, _COLS[i], :]
    return ang


def _pack_core_input(ang, pat_row, c):
    packed = np.zeros((_P, 81), np.float32)
    for b in range(_BL):
        packed[b * 32:b * 32 + _Q, 0:24] = ang[c * _BL + b]
    packed[:, 25:81] = pat_row  # col 24 stays zero (alpha_6 pad)
    return packed


def kernel(x, weights):
    global LAST_EXEC_TIME_NS, LAST_RESULTS
    from concourse.bass_utils import run_bass_kernel_spmd

    x = np.ascontiguousarray(np.asarray(x, np.float32))
    w = np.ascontiguousarray(np.asarray(weights, np.float32))

    if "nc" not in _CACHE:
        _CACHE["nc"] = _build_nc()
        _CACHE["pat"] = _pattern_input()
    nc = _CACHE["nc"]
    pat_row = _CACHE["pat"]

    ang = _pack_angles(x, w)  # (B, Q, 24)
    in_maps = [
        {"inp": _pack_core_input(ang, pat_row, c)} for c in range(_NCORES)
    ]

    trace = os.environ.get("KERNEL_TRACE", "0") == "1"
    res = run_bass_kernel_spmd(nc, in_maps, list(range(_NCORES)), trace=trace)
    LAST_EXEC_TIME_NS = res.exec_time_ns
    LAST_RESULTS = res

    state = np.empty((_B, _Q, 2), np.complex64)
    O = np.empty((_B, 1, 1), np.complex64)
    for c in range(_NCORES):
        o = np.asarray(res.results[c]["outp"], np.float32)  # (128, 5)
        for b in range(_BL):
            st = o[b * 32:b * 32 + _Q, 0:4].reshape(_Q, 2, 2)
            state[c * _BL + b] = st[..., 0] + 1j * st[..., 1]
            O[c * _BL + b, 0, 0] = np.complex64(o[b * 32, 4])

    return state.reshape(_B, _Q, 1, 2, 1), O
